# revision 1
# baseline (speedup 1.0000x reference)
"""Trainium2 Bass kernel for nn_DfOpStrided — v3 (full 128-partition packing).

Math (reference):
    x[t] = spec[:, 0, t, :96, :]                     (complex, [T, 96])
    spec_f[t] = sum_k c[t, k] * x[t + k - 4]         (complex MAC, zero-pad t<0)
    out[t] = alpha[t] * spec_f[t] + (1 - alpha[t]) * x[t]

Host-side fusion: out[t] = sum_k chat[t, k] * x[t+k-4] with
    chat[t, k] = alpha[t] * c[t, k]  (+ (1-alpha[t]) on Re(chat[t, 4]))
because tap k=4 multiplies x[t] itself.

Gauss 3-mult complex multiply per tap (host precomputes the combos):
    A = sum_k cr_k * s_k        (s = xr + xi, shifted per tap)
    B = sum_k (cr_k + ci_k) * xi_k
    C = sum_k (ci_k - cr_k) * xr_k
    re = A - B, im = A + C      (combined on host from the shipped A|B|C)

Packing: each worker row is one (batch, time-segment, freq-bin) triple with a
4-sample halo, Wt=500 samples per segment -> 2*4*96 = 768 rows per core = six
full 128-partition tiles. All DVE ops and DMAs use all 128 partitions.

fp16: DVE tensor_tensor runs 2x for 16-bit dtypes but needs 4B-aligned
operand starts, so each x row also carries a copy shifted by one element;
odd taps read the shifted copy at an even element offset. Tap products and
accumulation are stored fp16 (the DVE ALU computes in fp32 internally).

Pure data-parallel over batch: 16 batches -> 8 cores x 2 each.
"""

import sys

sys.path.insert(0, "/opt/trn_rl_repo")

import numpy as np
from concourse import bass, bacc, tile, mybir
from concourse.bass_utils import run_bass_kernel_spmd

B, T, F, NDF, ORDER = 16, 2000, 481, 96, 5
NCORES = 8
BPC = B // NCORES  # batches per core
PAD = ORDER - 1  # causal zero-pad
Wt = 500  # samples per row segment
SEG = T // Wt  # segments per batch
XWT = Wt + PAD  # x row width (halo)
ROWS = BPC * SEG * NDF  # 768 worker rows per core
P = 128
NT = ROWS // P  # 6 tiles per core

USE_FP16 = True
NPAR = 2 if USE_FP16 else 1

_cache: dict = {}


def _build():
    if "nc" in _cache:
        return _cache["nc"]
    f32 = mybir.dt.float32
    dt = mybir.dt.float16 if USE_FP16 else f32
    nc = bacc.Bacc("TRN2", target_bir_lowering=False, debug=False, num_devices=NCORES)
    # x rows: per row [par][j][XWT] with planes j: 0=s, 1=xi, 2=xr
    xin = nc.dram_tensor("xin", [NT, P, NPAR * 3 * XWT], dt, kind="ExternalInput")
    # coef rows: per (tile, tap): [P, 2*Wt] with planes j: 0=2*cr, 1=cr+ci
    # (third plane ci-cr = (cr+ci) - 2*cr is one plain subtract on device;
    # the A-chain then accumulates 2*A, halved on the host in the combine)
    coef = nc.dram_tensor("coef", [NT, ORDER, P, 2 * Wt], dt, kind="ExternalInput")
    # y rows: raw accumulator [A | B | C]
    y = nc.dram_tensor("y", [NT, P, 3 * Wt], dt, kind="ExternalOutput")

    with tile.TileContext(nc) as tc:
        with (
            tc.tile_pool(name="xp", bufs=3) as xpool,
            tc.tile_pool(name="cp", bufs=8) as cpool,
            tc.tile_pool(name="tp", bufs=4) as tpool,
            tc.tile_pool(name="ap", bufs=3) as apool,
        ):
            for i in range(NT):
                xt = xpool.tile([P, NPAR * 3 * XWT], dt, tag="xall")
                nc.scalar.dma_start(out=xt[:, :], in_=xin[i, :, :])
                cks = []
                for k in range(ORDER):
                    ckt = cpool.tile([P, 3 * Wt], dt, tag=f"ck{k % 2}")
                    eng = nc.sync if k % 2 == 0 else nc.gpsimd
                    eng.dma_start(out=ckt[:, : 2 * Wt], in_=coef[i, k, :, :])
                    # cd = cs - p  (p = 2*cr) — plain TT subtract runs 2x
                    nc.vector.tensor_tensor(
                        ckt[:, 2 * Wt : 3 * Wt],
                        ckt[:, Wt : 2 * Wt],
                        ckt[:, 0:Wt],
                        mybir.AluOpType.subtract,
                    )
                    cks.append(ckt)
                acc = None
                for k in range(ORDER):
                    par = k % 2 if USE_FP16 else 0
                    off = k - par  # even: 0,0,2,2,4
                    xv = xt[:, par * 3 * XWT : (par + 1) * 3 * XWT].rearrange(
                        "p (j n) -> p j n", j=3
                    )[:, :, off : off + Wt]
                    ck3 = cks[k][:, :].rearrange("p (j n) -> p j n", j=3)
                    if k == 0:
                        acc = apool.tile([P, 3 * Wt], dt, tag="acc")
                        a3 = acc[:, :].rearrange("p (j n) -> p j n", j=3)
                        nc.vector.tensor_tensor(a3, ck3, xv, mybir.AluOpType.mult)
                    else:
                        t = tpool.tile([P, 3 * Wt], dt, tag="t")
                        t3 = t[:, :].rearrange("p (j n) -> p j n", j=3)
                        nc.vector.tensor_tensor(t3, ck3, xv, mybir.AluOpType.mult)
                        acc2 = apool.tile([P, 3 * Wt], dt, tag="acc")
                        nc.vector.tensor_tensor(
                            acc2[:, :], acc[:, :], t[:, :], mybir.AluOpType.add
                        )
                        acc = acc2
                nc.scalar.dma_start(out=y[i, :, :], in_=acc[:, :])
    nc.compile()
    _cache["nc"] = nc
    return nc


def _host_prep(spec, coefs, alpha):
    """Build per-core xin/coef row arrays (all cores at once).

    Returns xin_all [NCORES, NT, P, NPAR*3*XWT], coef_all [NCORES, NT, ORDER,
    P, 3*Wt].
    """
    spec = np.asarray(spec, dtype=np.float32)
    coefs = np.asarray(coefs, dtype=np.float32)
    alpha = np.asarray(alpha, dtype=np.float32)
    dt = np.float16 if USE_FP16 else np.float32

    x = spec[:, 0, :, :NDF, :]  # [B, T, 96, 2]
    xr = x[..., 0].transpose(0, 2, 1)  # [B, 96, T] (views fine)
    xi = x[..., 1].transpose(0, 2, 1)

    # padded planes [B, 3, 96, PAD + T + 1] (one trailing col for parity-1)
    planes = np.zeros((B, 3, NDF, PAD + T + 1), dtype=np.float32)
    planes[:, 0, :, PAD : PAD + T] = xr + xi
    planes[:, 1, :, PAD : PAD + T] = xi
    planes[:, 2, :, PAD : PAD + T] = xr
    planes = planes.astype(dt)

    # x rows: [B, SEG, 96, npar, 3, XWT] -> flat rows (b, seg, f)
    xrow = np.empty((B, SEG, NDF, NPAR, 3, XWT), dtype=dt)
    for s in range(SEG):
        c0 = s * Wt
        for par in range(NPAR):
            sl = planes[:, :, :, c0 + par : c0 + par + XWT]  # [B, 3, 96, XWT]
            xrow[:, s, :, par] = sl.transpose(0, 2, 1, 3)
    xin_all = xrow.reshape(NCORES, NT, P, NPAR * 3 * XWT)

    a = alpha[:, :, 0]  # [B, T]
    ca = coefs * a[:, :, None, None, None]  # [B, T, 5, 96, 2]
    ca[:, :, ORDER - 1, :, 0] += (1.0 - a)[:, :, None]
    cr = np.ascontiguousarray(ca[..., 0].transpose(0, 2, 3, 1))  # [B, 5, 96, T]
    ci = np.ascontiguousarray(ca[..., 1].transpose(0, 2, 3, 1))

    comb = np.empty((2, B, ORDER, NDF, T), dtype=dt)
    comb[0] = 2.0 * cr
    comb[1] = cr + ci

    # coef rows: [B, SEG, 96, ORDER, 2, Wt] -> [B*SEG*96, ORDER, 2*Wt]
    crow = np.empty((B, SEG, NDF, ORDER, 2, Wt), dtype=dt)
    for s in range(SEG):
        c0 = s * Wt
        sl = comb[:, :, :, :, c0 : c0 + Wt]  # [2, B, ORDER, 96, Wt]
        crow[:, s] = sl.transpose(1, 3, 2, 0, 4)
    # rows grouped into tiles of 128, tap-major per tile
    coef_all = (
        crow.reshape(NCORES, NT, P, ORDER, 2 * Wt)
        .transpose(0, 1, 3, 2, 4)
        .copy()
    )
    return xin_all, coef_all


def kernel(spec, coefs, alpha, _bass_results_hook=None):
    nc = _build()
    xin_all, coef_all = _host_prep(spec, coefs, alpha)

    core_ids = list(range(NCORES))
    in_maps = [
        {"xin": xin_all[c], "coef": coef_all[c]} for c in core_ids
    ]
    res = run_bass_kernel_spmd(nc, in_maps, core_ids)
    if _bass_results_hook is not None:
        _bass_results_hook(res)

    yy = np.stack([res.results[c]["y"] for c in core_ids])  # [NC, NT, P, 3*Wt]
    abc = yy.reshape(B, SEG, NDF, 3, Wt).astype(np.float32)
    half_a = 0.5 * abc[:, :, :, 0]  # shipped A-plane is 2*A
    re = half_a - abc[:, :, :, 1]  # [B, SEG, 96, Wt]
    im = half_a + abc[:, :, :, 2]
    re = re.transpose(0, 2, 1, 3).reshape(B, NDF, T)
    im = im.transpose(0, 2, 1, 3).reshape(B, NDF, T)
    out = np.array(spec, dtype=np.float32, copy=True)
    out[:, 0, :, :NDF, 0] = re.transpose(0, 2, 1)
    out[:, 0, :, :NDF, 1] = im.transpose(0, 2, 1)
    return out

